# revision 7
# baseline (speedup 1.0000x reference)
"""Trainium2 Bass kernel for nn_CrossAttention_64218351009907.

Computation (reference):
    q   = (x @ Wq).reshape(B, H, E) / E**0.25
    dot = einsum("bhe,ne->bhn", q, keys) + values
    p   = softmax(dot, axis=-1)
    out = softmax(p * exp(curiosity)[:, None, None], axis=-1).mean(axis=1)

Strategy: data-parallel over batch across 8 NeuronCores (512 rows each).
Per core:
  - m1: qT[eh, b] = sum_k W'[k, eh] * xT[k, b]  (W' = Wq * E**-0.25),
    computed as 3-term f32r-compensated matmuls (W ~= W1 + Wr, x ~= x1 + xr,
    terms W1*x1 + Wr*x1 + W1*xr) -> near-fp32 accuracy at 1 cycle/row.
  - q split on device: qf = rne11(q) (DVE rounds on f32r write),
    qres = q - qf stored bf16.
  - m2: dot[b, n] = qf*k1 + qf*kr + qres*kb (+ values via K=1 matmuls, split
    v1+vr packed into PE row strips 0/32), accumulated in PSUM fp32.
  - softmax1: ACT exp with fused per-partition accumulate (s1).
  - softmax2: ACT exp with per-partition scale c = exp(cur)/s1, fused s2.
  - head mean: DVE out_acc += ex2 * (1/(16*s2)).
"""

import numpy as np

B, IN, H, E, NEP = 4096, 1024, 16, 256, 2048
NCORES = 8
BC = B // NCORES          # 512 batch rows per core
NBT = BC // 128           # 4 batch tiles per core
M1K = IN // 128           # 8 contraction chunks for m1
EHC = (E * H) // 128      # 32 eh chunks (q rows)
KC = E // 128             # 2 contraction chunks for m2
NNJ = NEP // 512          # 4 psum bank chunks per dot tile
E4 = float(E) ** 0.25

_CACHE = {}


def _rne11(x):
    """Round-to-nearest-even keeping 11 explicit mantissa bits (= f32r)."""
    x = np.ascontiguousarray(x, dtype=np.float32)
    b = x.view(np.uint32).astype(np.uint64)
    bias = ((b >> 12) & 1) + (1 << 11) - 1
    out = (((b + bias) >> 12) << 12).astype(np.uint32)
    return out.view(np.float32).reshape(x.shape)


def _build(n_reps=1, n_bt=NBT, n_h=H, use_values=2, inplace_exp2=True,
           use_m1=True, use_accum=True):
    import concourse.bacc as bacc
    import concourse.tile as tile
    import concourse.mybir as mybir

    f32 = mybir.dt.float32
    f32r = mybir.dt.float32r
    bf16 = mybir.dt.bfloat16
    Exp = mybir.ActivationFunctionType.Exp
    mult = mybir.AluOpType.mult
    sub = mybir.AluOpType.subtract
    add = mybir.AluOpType.add

    nc = bacc.Bacc(trn_type="TRN2", target_bir_lowering=False, debug=False)

    dx1 = nc.dram_tensor("x1", [IN, BC], f32, kind="ExternalInput")
    dxr = nc.dram_tensor("xr", [IN, BC], f32, kind="ExternalInput")
    dw1 = nc.dram_tensor("w1", [IN, E * H], f32, kind="ExternalInput")
    dwr = nc.dram_tensor("wr", [IN, E * H], f32, kind="ExternalInput")
    dk1 = nc.dram_tensor("k1", [E, NEP], f32, kind="ExternalInput")
    dkr = nc.dram_tensor("kr", [E, NEP], f32, kind="ExternalInput")
    dkb = nc.dram_tensor("kb", [E, NEP], bf16, kind="ExternalInput")
    dv2 = nc.dram_tensor("v2", [2, NEP], f32, kind="ExternalInput")
    don = nc.dram_tensor("ones2", [2, 128], f32, kind="ExternalInput")
    dcur = nc.dram_tensor("cur", [128, NBT], f32, kind="ExternalInput")
    dout = nc.dram_tensor("out", [BC, NEP], f32, kind="ExternalOutput")

    with tile.TileContext(nc) as tc:
        with tc.tile_pool(name="consts", bufs=1) as csp, \
             tc.tile_pool(name="qf", bufs=1) as qfp, \
             tc.tile_pool(name="qr", bufs=1) as qrp, \
             tc.tile_pool(name="scal", bufs=24) as scal:
            for _rep in range(n_reps):
                ones_t = csp.tile([2, 128], f32r, name="ones_t")
                nc.sync.dma_start(ones_t[:], don[:].bitcast(f32r))
                va_t = csp.tile([2, NEP], f32r, name="va_t")
                nc.sync.dma_start(va_t[:], dv2[:].bitcast(f32r))
                cur_t = csp.tile([128, NBT], f32, name="cur_t")
                nc.sync.dma_start(cur_t[:], dcur[:])

                # ---------------- phase 1: m1 + q split ----------------
                qf = []
                qr = []
                with tc.tile_pool(name="xp", bufs=1) as xp, \
                     tc.tile_pool(name="wp", bufs=32) as wp, \
                     tc.tile_pool(name="mps", bufs=4, space="PSUM") as mps:
                    x1_t = []
                    xr_t = []
                    for k in range(M1K):
                        t1 = xp.tile([128, BC], f32r, name=f"x1_{k}")
                        nc.sync.dma_start(
                            t1[:], dx1[k * 128:(k + 1) * 128, :].bitcast(f32r))
                        x1_t.append(t1)
                        t2 = xp.tile([128, BC], f32r, name=f"xr_{k}")
                        nc.sync.dma_start(
                            t2[:], dxr[k * 128:(k + 1) * 128, :].bitcast(f32r))
                        xr_t.append(t2)
                    for m in range(EHC):
                        ps = mps.tile([128, BC], f32, name="qpsum")
                        n_mm = M1K * 3
                        i_mm = 0
                        for k in range(M1K):
                            w1t = wp.tile([128, 128], f32r, name="w1t")
                            nc.sync.dma_start(
                                w1t[:],
                                dw1[k * 128:(k + 1) * 128,
                                    m * 128:(m + 1) * 128].bitcast(f32r))
                            wrt = wp.tile([128, 128], f32r, name="wrt")
                            nc.sync.dma_start(
                                wrt[:],
                                dwr[k * 128:(k + 1) * 128,
                                    m * 128:(m + 1) * 128].bitcast(f32r))
                            for lhs, rhs in ((w1t, x1_t[k]), (w1t, xr_t[k]),
                                             (wrt, x1_t[k])):
                                nc.tensor.matmul(
                                    ps[:], lhs[:], rhs[:],
                                    start=(i_mm == 0), stop=(i_mm == n_mm - 1))
                                i_mm += 1
                        q_t = qfp.tile([128, BC], f32r, name=f"qf{m}")
                        nc.vector.tensor_copy(q_t[:], ps[:])  # rounds to f32r
                        qr_t = qrp.tile([128, BC], bf16, name=f"qr{m}")
                        nc.vector.scalar_tensor_tensor(
                            qr_t[:], ps[:], 1.0, q_t[:], op0=mult, op1=sub)
                        qf.append(q_t)
                        qr.append(qr_t)

                # ---------------- phase 2: attention + double softmax ----
                with tc.tile_pool(name="kt", bufs=1) as ktp, \
                     tc.tile_pool(name="dps", bufs=2, space="PSUM") as dps, \
                     tc.tile_pool(name="exp", bufs=3) as exp_pool, \
                     tc.tile_pool(name="oacc", bufs=2) as opool:
                    k1_t = []
                    kr_t = []
                    kb_t = []
                    for kc in range(KC):
                        a = ktp.tile([128, NEP], f32r, name=f"k1_{kc}")
                        nc.sync.dma_start(
                            a[:], dk1[kc * 128:(kc + 1) * 128, :].bitcast(f32r))
                        k1_t.append(a)
                        b_ = ktp.tile([128, NEP], f32r, name=f"kr_{kc}")
                        nc.sync.dma_start(
                            b_[:], dkr[kc * 128:(kc + 1) * 128, :].bitcast(f32r))
                        kr_t.append(b_)
                        c_ = ktp.tile([128, NEP], bf16, name=f"kb_{kc}")
                        nc.sync.dma_start(c_[:], dkb[kc * 128:(kc + 1) * 128, :])
                        kb_t.append(c_)

                    def stage_a(bt, h):
                        ps = dps.tile([128, NEP], f32, name="dot")
                        for nj in range(NNJ):
                            sl = slice(nj * 512, (nj + 1) * 512)
                            bsl = slice(bt * 128, (bt + 1) * 128)
                            mms = []
                            for kc in range(KC):
                                eh = h * KC + kc
                                mms += [(qf[eh][:, bsl], k1_t[kc][:, sl]),
                                        (qf[eh][:, bsl], kr_t[kc][:, sl]),
                                        (qr[eh][:, bsl], kb_t[kc][:, sl])]
                            if use_values == 1:
                                mms.append((ones_t[0:1, :], va_t[0:1, sl]))
                            elif use_values >= 2:
                                mms.append((ones_t[:, :], va_t[:, sl]))
                            for i_mm, (lhs, rhs) in enumerate(mms):
                                nc.tensor.matmul(
                                    ps[:, sl], lhs, rhs,
                                    start=(i_mm == 0),
                                    stop=(i_mm == len(mms) - 1))
                        ex1 = exp_pool.tile([128, NEP], f32, name="ex1")
                        s1 = scal.tile([128, 1], f32, name="s1")
                        nc.scalar.activation(ex1[:], ps[:], Exp, accum_out=s1[:])
                        rs1 = scal.tile([128, 1], f32, name="rs1")
                        nc.vector.reciprocal(rs1[:], s1[:])
                        c = scal.tile([128, 1], f32, name="c")
                        nc.vector.tensor_tensor(
                            c[:], rs1[:], cur_t[:, bt:bt + 1], op=mult)
                        return ex1, c

                    def stage_b(bt, h, ex1, c, oat):
                        s2 = scal.tile([128, 1], f32, name="s2")
                        if inplace_exp2:
                            ex2 = ex1
                        else:
                            ex2 = exp_pool.tile([128, NEP], f32, name="ex2",
                                                bufs=2)
                        nc.scalar.activation(ex2[:], ex1[:], Exp, scale=c[:],
                                             accum_out=s2[:])
                        rs2 = scal.tile([128, 1], f32, name="rs2")
                        nc.vector.reciprocal(rs2[:], s2[:])
                        w = scal.tile([128, 1], f32, name="w")
                        nc.vector.tensor_scalar_mul(w[:], rs2[:], 1.0 / H)
                        if h == 0:
                            nc.vector.tensor_scalar(
                                oat[:], ex2[:], w[:], None, op0=mult)
                        else:
                            nc.vector.scalar_tensor_tensor(
                                oat[:], ex2[:], w[:], oat[:], op0=mult, op1=add)
                        if h == n_h - 1:
                            nc.sync.dma_start(
                                dout[bt * 128:(bt + 1) * 128, :], oat[:])

                    items = [(bt, h) for bt in range(n_bt) for h in range(n_h)]
                    oat_by_bt = {}
                    pend = None
                    for bt, h in items:
                        if h == 0:
                            oat_by_bt[bt] = opool.tile([128, NEP], f32,
                                                       name="oacc")
                        cur_res = stage_a(bt, h)
                        if pend is not None:
                            stage_b(*pend)
                        pend = (bt, h, cur_res[0], cur_res[1], oat_by_bt[bt])
                    stage_b(*pend)

    nc.compile()
    return nc


def _prep_inputs(x, curiosity_score, Wq, keys, values):
    """Host-side sharding + operand splitting. Returns per-core input maps."""
    x = np.ascontiguousarray(np.asarray(x, dtype=np.float32))
    cur = np.asarray(curiosity_score, dtype=np.float32)
    Wq = np.asarray(Wq, dtype=np.float32)
    keys = np.asarray(keys, dtype=np.float32)
    values = np.asarray(values, dtype=np.float32)

    import ml_dtypes

    W = (Wq * np.float32(1.0 / E4)).astype(np.float32)
    W1 = _rne11(W)
    Wr = (W - W1).astype(np.float32)
    kT = np.ascontiguousarray(keys.T)               # [E, NEP]
    k1 = _rne11(kT)
    kr = (kT - k1).astype(np.float32)
    kb = kT.astype(ml_dtypes.bfloat16)
    v = values.reshape(1, NEP).astype(np.float32)
    v1 = _rne11(v)
    v2 = np.concatenate([v1, v - v1], axis=0).astype(np.float32)
    ones2 = np.ones((2, 128), dtype=np.float32)
    cur_e = np.exp(cur).astype(np.float32)

    in_maps = []
    for c in range(NCORES):
        xc = x[c * BC:(c + 1) * BC]                 # [BC, IN]
        xT = np.ascontiguousarray(xc.T)             # [IN, BC]
        x1 = _rne11(xT)
        xr = (xT - x1).astype(np.float32)
        cur_c = np.ascontiguousarray(
            cur_e[c * BC:(c + 1) * BC].reshape(NBT, 128).T)   # [128, NBT]
        in_maps.append({
            "x1": x1, "xr": xr, "w1": W1, "wr": Wr,
            "k1": k1, "kr": kr, "kb": kb, "v2": v2,
            "ones2": ones2, "cur": cur_c,
        })
    return in_maps


def kernel(x, curiosity_score, Wq, keys, values):
    from concourse.bass_utils import run_bass_kernel_spmd

    if "nc" not in _CACHE:
        _CACHE["nc"] = _build()
    nc = _CACHE["nc"]
    in_maps = _prep_inputs(x, curiosity_score, Wq, keys, values)
    res = run_bass_kernel_spmd(nc, in_maps, list(range(NCORES)))
    out = np.concatenate([res.results[c]["out"] for c in range(NCORES)], axis=0)
    return out.astype(np.float32)


# revision 12
# speedup vs baseline: 14927.7914x; 14927.7914x over previous
"""Trainium2 Bass kernel for nn_CrossAttention_64218351009907.

Computation (reference):
    q   = (x @ Wq).reshape(B, H, E) / E**0.25
    dot = einsum("bhe,ne->bhn", q, keys) + values
    p   = softmax(dot, axis=-1)
    out = softmax(p * exp(curiosity)[:, None, None], axis=-1).mean(axis=1)

Data-parallel over batch across 8 NeuronCores (512 rows each). Per core:
  - m1: qT[eh, b] = sum W'[k, eh] * xT[k, b]  (W' = Wq * E**-0.25), as 3-term
    split-compensated matmuls (near-fp32 accuracy at 1 cycle/row).
  - q split on device into (rounded, residual) pair for m2.
  - m2: dot = q*k via 3-term split-compensated matmuls into PSUM fp32.
  - softmax1: ACT exp; DVE fused (exp * exp(values)-broadcast) + row-sum s1.
  - softmax2: ACT exp with per-partition scale c = exp(cur)/s1, fused sum s2.
  - head mean: DVE out_acc += ex2 * (1/(16*s2)).
"""

import numpy as np

B, IN, H, E, NEP = 4096, 1024, 16, 256, 2048
NCORES = 8
BC = B // NCORES          # 512 batch rows per core
NBT = BC // 128           # 4 batch tiles per core
M1K = IN // 128           # 8 contraction chunks for m1
EHC = (E * H) // 128      # 32 eh chunks (q rows)
KC = E // 128             # 2 contraction chunks for m2
NNJ = NEP // 512          # 4 psum bank chunks per dot tile
E4 = float(E) ** 0.25

_CACHE = {}


def _rne11(x):
    """Round-to-nearest-even keeping 11 explicit mantissa bits (= f32r)."""
    x = np.ascontiguousarray(x, dtype=np.float32)
    b = x.view(np.uint32).astype(np.uint64)
    bias = ((b >> 12) & 1) + (1 << 11) - 1
    out = (((b + bias) >> 12) << 12).astype(np.uint32)
    return out.view(np.float32).reshape(x.shape)


def _build(n_reps=1, n_bt=NBT, n_h=H, m1_terms=3, m2_terms=3,
           dtype="bf16", values_on="pe", skip_softmax=False, skip_m1=False):
    import concourse.bacc as bacc
    import concourse.tile as tile
    import concourse.mybir as mybir

    f32 = mybir.dt.float32
    bf16 = mybir.dt.bfloat16
    f32r = mybir.dt.float32r
    mmdt = f32r if dtype == "f32r" else bf16
    resdt = bf16
    iodt = f32 if dtype == "f32r" else bf16
    Exp = mybir.ActivationFunctionType.Exp
    mult = mybir.AluOpType.mult
    sub = mybir.AluOpType.subtract
    add = mybir.AluOpType.add

    def bc(ap):
        return ap.bitcast(mmdt) if dtype == "f32r" else ap

    nc = bacc.Bacc(trn_type="TRN2", target_bir_lowering=False, debug=False)

    dx1 = nc.dram_tensor("x1", [IN, BC], iodt, kind="ExternalInput")
    dxr = nc.dram_tensor("xr", [IN, BC], iodt, kind="ExternalInput")
    dw1 = nc.dram_tensor("w1", [IN, E * H], iodt, kind="ExternalInput")
    dwr = nc.dram_tensor("wr", [IN, E * H], iodt, kind="ExternalInput")
    dk1 = nc.dram_tensor("k1", [E, NEP], iodt, kind="ExternalInput")
    dkr = nc.dram_tensor("kr", [E, NEP], iodt, kind="ExternalInput")
    dkb = nc.dram_tensor("kb", [E, NEP], bf16, kind="ExternalInput")
    dv2 = nc.dram_tensor("v2", [2, NEP], f32, kind="ExternalInput")
    don = nc.dram_tensor("ones2", [2, 128], f32, kind="ExternalInput")
    dev = nc.dram_tensor("ev", [128, NEP], f32, kind="ExternalInput")
    dcur = nc.dram_tensor("cur", [128, NBT], f32, kind="ExternalInput")
    dout = nc.dram_tensor("out", [BC, NEP], f32, kind="ExternalOutput")

    with tile.TileContext(nc) as tc:
        with tc.tile_pool(name="consts", bufs=1) as csp, \
             tc.tile_pool(name="qf", bufs=1) as qfp, \
             tc.tile_pool(name="qr", bufs=1) as qrp, \
             tc.tile_pool(name="scal", bufs=24) as scal:
            for _rep in range(n_reps):
                if values_on == "pe":
                    ones_t = csp.tile([2, 128], f32r, name="ones_t")
                    nc.sync.dma_start(ones_t[:], don[:].bitcast(f32r))
                    va_t = csp.tile([2, NEP], f32r, name="va_t")
                    nc.sync.dma_start(va_t[:], dv2[:].bitcast(f32r))
                else:
                    ev_t = csp.tile([128, NEP], f32, name="ev_t")
                    nc.sync.dma_start(ev_t[:], dev[:])
                cur_t = csp.tile([128, NBT], f32, name="cur_t")
                nc.sync.dma_start(cur_t[:], dcur[:])

                # ---------------- phase 1: m1 + q split ----------------
                qf = []
                qr = []
                if skip_m1:
                    for m in range(EHC):
                        q_t = qfp.tile([128, BC], mmdt, name=f"qf{m}")
                        nc.sync.dma_start(
                            q_t[:],
                            bc(dx1[(m % M1K) * 128:(m % M1K + 1) * 128, :]))
                        qr_t = qrp.tile([128, BC], resdt, name=f"qr{m}")
                        nc.vector.tensor_copy(qr_t[:], q_t[:])
                        qf.append(q_t)
                        qr.append(qr_t)
                else:
                  with tc.tile_pool(name="xp", bufs=1) as xp, \
                       tc.tile_pool(name="wp", bufs=3) as wp, \
                       tc.tile_pool(name="mps", bufs=8, space="PSUM") as mps:
                    x1_t = []
                    xr_t = []
                    for k in range(M1K):
                        t1 = xp.tile([128, BC], mmdt, name=f"x1_{k}")
                        nc.sync.dma_start(t1[:],
                                          bc(dx1[k * 128:(k + 1) * 128, :]))
                        x1_t.append(t1)
                        if m1_terms >= 2:
                            t2 = xp.tile([128, BC], mmdt, name=f"xr_{k}")
                            nc.sync.dma_start(
                                t2[:], bc(dxr[k * 128:(k + 1) * 128, :]))
                            xr_t.append(t2)
                    for mg in range(EHC // 4):     # groups of 4 eh-chunks
                        pss = [mps.tile([128, BC], f32, name="qpsum")
                               for _ in range(4)]
                        n_mm = M1K * m1_terms
                        i_mm = 0
                        for k in range(M1K):
                            w1t = wp.tile([128, 512], mmdt, name="w1t")
                            nc.sync.dma_start(
                                w1t[:],
                                bc(dw1[k * 128:(k + 1) * 128,
                                       mg * 512:(mg + 1) * 512]))
                            pairs = [(w1t, x1_t[k])]
                            if m1_terms >= 2:
                                pairs.append((w1t, xr_t[k]))
                            if m1_terms >= 3:
                                wrt = wp.tile([128, 512], mmdt, name="wrt")
                                nc.sync.dma_start(
                                    wrt[:],
                                    bc(dwr[k * 128:(k + 1) * 128,
                                           mg * 512:(mg + 1) * 512]))
                                pairs.append((wrt, x1_t[k]))
                            for lhs, rhs in pairs:
                                for mj in range(4):
                                    nc.tensor.matmul(
                                        pss[mj][:],
                                        lhs[:, mj * 128:(mj + 1) * 128],
                                        rhs[:], start=(i_mm == 0),
                                        stop=(i_mm == n_mm - 1))
                                i_mm += 1
                        for mj in range(4):
                            m = mg * 4 + mj
                            ps = pss[mj]
                            q_t = qfp.tile([128, BC], mmdt, name=f"qf{m}")
                            nc.vector.tensor_copy(q_t[:], ps[:])
                            qr_t = qrp.tile([128, BC], resdt, name=f"qr{m}")
                            nc.vector.scalar_tensor_tensor(
                                qr_t[:], ps[:], 1.0, q_t[:], op0=mult, op1=sub)
                            qf.append(q_t)
                            qr.append(qr_t)

                # ---------------- phase 2: attention + double softmax ----
                with tc.tile_pool(name="kt", bufs=1) as ktp, \
                     tc.tile_pool(name="dps", bufs=2, space="PSUM") as dps, \
                     tc.tile_pool(name="exp", bufs=3) as exp_pool, \
                     tc.tile_pool(name="oacc", bufs=2) as opool:
                    k1_t = []
                    kr_t = []
                    kb_t = []
                    for kc in range(KC):
                        a = ktp.tile([128, NEP], mmdt, name=f"k1_{kc}")
                        nc.sync.dma_start(a[:],
                                          bc(dk1[kc * 128:(kc + 1) * 128, :]))
                        k1_t.append(a)
                        if m2_terms >= 2:
                            b_ = ktp.tile([128, NEP], mmdt, name=f"kr_{kc}")
                            nc.sync.dma_start(
                                b_[:], bc(dkr[kc * 128:(kc + 1) * 128, :]))
                            kr_t.append(b_)
                        if m2_terms >= 3:
                            c_ = ktp.tile([128, NEP], bf16, name=f"kb_{kc}")
                            nc.sync.dma_start(
                                c_[:], dkb[kc * 128:(kc + 1) * 128, :])
                            kb_t.append(c_)

                    def stage_a(bt, h):
                        ps = dps.tile([128, NEP], f32, name="dot")
                        bsl = slice(bt * 128, (bt + 1) * 128)
                        for nj in range(NNJ):
                            sl = slice(nj * 512, (nj + 1) * 512)
                            mms = []
                            for kc in range(KC):
                                eh = h * KC + kc
                                mms.append((qf[eh][:, bsl], k1_t[kc][:, sl]))
                                if m2_terms >= 2:
                                    mms.append((qf[eh][:, bsl],
                                                kr_t[kc][:, sl]))
                                if m2_terms >= 3:
                                    mms.append((qr[eh][:, bsl],
                                                kb_t[kc][:, sl]))
                            if values_on == "pe":
                                mms.append((ones_t[:, :], va_t[:, sl]))
                            for i_mm, (lhs, rhs) in enumerate(mms):
                                nc.tensor.matmul(
                                    ps[:, sl], lhs, rhs, start=(i_mm == 0),
                                    stop=(i_mm == len(mms) - 1))
                        if skip_softmax:
                            return None, None
                        ex1 = exp_pool.tile([128, NEP], f32, name="ex1")
                        s1 = scal.tile([128, 1], f32, name="s1")
                        if values_on == "pe":
                            nc.scalar.activation(ex1[:], ps[:], Exp,
                                                 accum_out=s1[:])
                        else:
                            nc.scalar.activation(ex1[:], ps[:], Exp)
                            nc.vector.tensor_tensor_reduce(
                                ex1[:], ex1[:], ev_t[:], 1.0, 0.0,
                                op0=mult, op1=add, accum_out=s1[:])
                        rs1 = scal.tile([128, 1], f32, name="rs1")
                        nc.vector.reciprocal(rs1[:], s1[:])
                        c = scal.tile([128, 1], f32, name="c")
                        nc.vector.tensor_tensor(
                            c[:], rs1[:], cur_t[:, bt:bt + 1], op=mult)
                        return ex1, c

                    def stage_b(bt, h, ex1, c, oat):
                        if skip_softmax:
                            return
                        s2 = scal.tile([128, 1], f32, name="s2")
                        nc.scalar.activation(ex1[:], ex1[:], Exp, scale=c[:],
                                             accum_out=s2[:])
                        rs2 = scal.tile([128, 1], f32, name="rs2")
                        nc.vector.reciprocal(rs2[:], s2[:])
                        w = scal.tile([128, 1], f32, name="w")
                        nc.vector.tensor_scalar_mul(w[:], rs2[:], 1.0 / H)
                        if h == 0:
                            nc.vector.tensor_scalar(
                                oat[:], ex1[:], w[:], None, op0=mult)
                        else:
                            nc.vector.scalar_tensor_tensor(
                                oat[:], ex1[:], w[:], oat[:], op0=mult,
                                op1=add)
                        if h == n_h - 1:
                            nc.sync.dma_start(
                                dout[bt * 128:(bt + 1) * 128, :], oat[:])

                    items = [(bt, h) for bt in range(n_bt) for h in range(n_h)]
                    oat_by_bt = {}
                    pend = None
                    for bt, h in items:
                        if h == 0:
                            oat_by_bt[bt] = opool.tile([128, NEP], f32,
                                                       name="oacc")
                        cur_res = stage_a(bt, h)
                        if pend is not None:
                            stage_b(*pend)
                        pend = (bt, h, cur_res[0], cur_res[1], oat_by_bt[bt])
                    stage_b(*pend)

    nc.compile()
    return nc


def _prep_inputs(x, curiosity_score, Wq, keys, values, dtype="bf16"):
    """Host-side sharding + operand splitting. Returns per-core input maps."""
    x = np.ascontiguousarray(np.asarray(x, dtype=np.float32))
    cur = np.asarray(curiosity_score, dtype=np.float32)
    Wq = np.asarray(Wq, dtype=np.float32)
    keys = np.asarray(keys, dtype=np.float32)
    values = np.asarray(values, dtype=np.float32)

    import ml_dtypes

    if dtype == "f32r":
        def split(a):
            a1 = _rne11(a)
            return a1, (a - a1).astype(np.float32)
    else:
        def split(a):
            a1 = a.astype(ml_dtypes.bfloat16)
            a2 = (a - a1.astype(np.float32)).astype(ml_dtypes.bfloat16)
            return a1, a2

    W = (Wq * np.float32(1.0 / E4)).astype(np.float32)
    W1, Wr = split(W)
    kT = np.ascontiguousarray(keys.T)               # [E, NEP]
    k1, kr = split(kT)
    kb = kT.astype(ml_dtypes.bfloat16)
    v = values.reshape(1, NEP).astype(np.float32)
    v1 = _rne11(v)
    v2 = np.concatenate([v1, v - v1], axis=0).astype(np.float32)
    ones2 = np.ones((2, 128), dtype=np.float32)
    ev = np.ascontiguousarray(
        np.broadcast_to(np.exp(v.astype(np.float64)).astype(np.float32),
                        (128, NEP)))
    cur_e = np.exp(cur).astype(np.float32)

    in_maps = []
    for c in range(NCORES):
        xc = x[c * BC:(c + 1) * BC]
        xT = np.ascontiguousarray(xc.T)             # [IN, BC]
        x1, xr = split(xT)
        cur_c = np.ascontiguousarray(
            cur_e[c * BC:(c + 1) * BC].reshape(NBT, 128).T)
        in_maps.append({
            "x1": x1, "xr": xr, "w1": W1, "wr": Wr,
            "k1": k1, "kr": kr, "kb": kb, "v2": v2,
            "ones2": ones2, "ev": ev, "cur": cur_c,
        })
    return in_maps


def kernel(x, curiosity_score, Wq, keys, values):
    from concourse.bass_utils import run_bass_kernel_spmd

    if "nc" not in _CACHE:
        _CACHE["nc"] = _build()
    nc = _CACHE["nc"]
    in_maps = _prep_inputs(x, curiosity_score, Wq, keys, values)
    res = run_bass_kernel_spmd(nc, in_maps, list(range(NCORES)))
    out = np.concatenate([res.results[c]["out"] for c in range(NCORES)],
                         axis=0)
    return out.astype(np.float32)
